# revision 27
# baseline (speedup 1.0000x reference)
"""MinGRU cell on 8 Trainium2 NeuronCores (Bass/Tile).

Math (per batch b, hidden h):
    gz = x @ W_z^T ; gh = x @ W_h^T                 (two GEMMs, K=D=1024)
    z  = sigmoid(gz + b_z)
    h_t = (1 - z_t) * h_{t-1} + z_t * (gh_t + b_h)  (affine scan over T)

Distribution: data-parallel over batch B=16 -> 2 batches per core, weights
replicated; no cross-core communication.

Per-core steady state: the PE streams the GEMMs (128 N=512 bf16 matmuls
per 512-token step, ~218 ns each = the bf16 roofline) with no other PE
work: the scan output is stored to DRAM in [H, T] layout straight from
the DVE scan tiles and transposed to [T, H] on the host, so the PE's
out-transposes (and their ACT/DVE copies and the 16 us drain tail) are
gone entirely. x^T tiles for steps 1..7 are produced by the DMA crossbar
(dma_start_transpose) straight from (host-precast bf16) DRAM, issued a
full step ahead — the crossbar's completion semaphore has been observed
to lead its data on profiled runs, so every crossbar transpose here has
>15 us between issue and first consumer. Step 0's x^T is built on the PE
instead (plain loads + tensor-engine transposes, j-outer so transposes
start as each x row-block lands). W arrives pre-transposed bf16 from the
host (weight pre-packing) in four half-H strided DMAs per W; the z/h
quarter-0 pair is split by dc-parity across BOTH HWDGE queues so the
first GEMMs wait on only ~250K per queue, quarters 1-3 stream whole
(W_z on SP, W_h on ACT), and step-0 GEMMs run in z/h quarter-pairs so
they track the W quarters as they land. Bias/h0 gathers go to the
GpSimd SWDGE queue. ACT runs the two sigmoids (z emitted before 1-z so
the DVE fuse starts an ACT-op earlier), DVE the (gh+b_h)*z fuse (bf16
out) and the affine scan. Output is written bf16 and upcast to f32 on
the host (it was computed in bf16 either way).

Measured notes from this optimization session (kept for future work):
- SBUF tile creation order is load-bearing: reordering the consts-pool
  tiles slowed every PE instruction ~20% (LDWEIGHTS/rhs SBUF line
  interaction). Keep x row-blocks created before the W quarters and the
  quarters interleaved z,h.
- dma_start is an engine instruction that can block the issuing engine
  on HWDGE queue-capacity semaphores: >6 large outstanding DMAs on one
  engine stalled it >10 us and starved everything emitted behind it
  (e.g. the prologue PSUM->SBUF copies, or the ACT sigmoids that drain
  GEMM PSUM banks). Never put a store's scan-wait on the ACT engine.
- The framework preamble (two all-engine barriers + queue MMIO setup)
  means no DMA moves before ~6-7 us; with ~140 GB/s per HWDGE queue and
  6 MB of W+x to land, the ~11 us of PE idle before steady state is
  bandwidth-bound, not ordering-bound.
- The HAM power-ramp limiter clamps PE util to 0.5 for ~7-10 us
  starting ~3 us after sustained matmul activity begins; junk warmup
  bursts (narrow or wide, 1-5 us) do not trigger it early, it always
  fires on the real GEMM stream. ~3 us of stretch is unavoidable.
- fp8 GEMMs are dead for this tolerance: e4m3 costs ~4e-2 rel err from
  either operand (gate is 2e-2), and error-compensated variants need
  >=2 fp8 passes per logical GEMM = no speedup over bf16. DoubleRow
  also needs K=256 chunks, so a partial-K fp8 split can't go below
  f=1/4, which is already at the gate.
"""

import sys

sys.path.insert(0, "/opt/trn_rl_repo")

from contextlib import ExitStack

import numpy as np
import ml_dtypes

import concourse.bass as bass
import concourse.mybir as mybir
import concourse.tile as tile
from concourse import bacc
from concourse.bass import ts, ds
from concourse.bass_utils import run_bass_kernel_spmd
from concourse.masks import make_identity

B, T, D, H = 16, 2048, 1024, 1024
NCORES = 8
B_LOC = B // NCORES  # 2
P = 128
TC = 512  # tokens per step
NSTEP = B_LOC * T // TC  # 8
NTC = T // TC  # 4 steps per batch
TSUB = TC // P  # 4
DC = D // P  # 8 contraction chunks
HC = H // P  # 8 hidden chunks
HQ = H // 4  # 256, one h-quarter of W per DMA

F32 = mybir.dt.float32
BF16 = mybir.dt.bfloat16
AF = mybir.ActivationFunctionType
OP = mybir.AluOpType

_CACHE = {}


def _mingru_tile(tc, out, x, h0, wzT, bz, whT, bh):
    nc = tc.nc

    with ExitStack() as ctx:
        consts = ctx.enter_context(tc.tile_pool(name="consts", bufs=1))

        id_bf = consts.tile([P, P], BF16)
        make_identity(nc, id_bf)

        # Small strided gathers on the otherwise idle SWDGE queue.
        bz_sb = consts.tile([P, HC], F32)
        nc.gpsimd.dma_start(out=bz_sb, in_=bz.rearrange("(c p) -> p c", p=P))
        bh_sb = consts.tile([P, HC], F32)
        nc.gpsimd.dma_start(out=bh_sb, in_=bh.rearrange("(c p) -> p c", p=P))
        hp_sb = consts.tile([P, B_LOC * HC], F32)
        nc.gpsimd.dma_start(out=hp_sb, in_=h0.rearrange("b (c p) -> p (b c)", p=P))
        nbz_sb = consts.tile([P, HC], F32)
        nc.vector.tensor_scalar_mul(nbz_sb, bz_sb, -1.0)

        xt_p = ctx.enter_context(tc.tile_pool(name="xt", bufs=2))
        azb_p = ctx.enter_context(tc.tile_pool(name="azb", bufs=2))
        scan_p = ctx.enter_context(tc.tile_pool(name="scan", bufs=2))
        xnat_p = ctx.enter_context(tc.tile_pool(name="xnat", bufs=1))

        def step_bt(s):
            return s // NTC, s % NTC

        # Step 0's x, natural layout, as two 2-row-block DMAs (even j's on
        # SP, odd on ACT) so both queues carry 512K of x each.
        xn_e = xnat_p.tile([P, 2, D], BF16, tag="xn_e", name="xn_e")
        xn_o = xnat_p.tile([P, 2, D], BF16, tag="xn_o", name="xn_o")
        x0v = x[0, ds(0, TC), :].rearrange("(pair j p) d -> p j pair d", j=2, p=P)
        nc.sync.dma_start(out=xn_e, in_=x0v[:, 0])
        nc.scalar.dma_start(out=xn_o, in_=x0v[:, 1])

        def xnt(j):
            return (xn_e if j % 2 == 0 else xn_o)[:, j // 2]

        # W^T arrives pre-transposed [D, H] bf16 from the host. One strided
        # DMA per h-quarter (512B row chunks):
        #   wt[wn][r][p, dc*HQ + h'] = W^T[dc*128 + p, r*HQ + h']
        # lhsT block (hc,dc) = wt[wn][hc//2][:, dc*HQ + (hc%2)*128 ...].
        # The startup is supply-bound (last GEMM ~= first GEMM + total work
        # + W-starvation stalls), so the z/h quarter-0 pair is split by
        # dc-parity across BOTH queues (first GEMMs wait on only ~250K per
        # queue) while quarters 1-3 stream whole, z->SP / h->ACT. That
        # keeps 6 large DMA issues per engine (7+ was measured to block
        # the issuing engine on queue-capacity semaphores).
        wt = {"z": [], "h": []}
        for r in range(4):
            for wn in ("z", "h"):
                wt[wn].append(consts.tile([P, DC * HQ], BF16, name=f"wt_{wn}{r}"))

        def w_issue(wn, r, eng, par=None):
            w_ap = wzT if wn == "z" else whT
            out_v = wt[wn][r].rearrange("p (dc h) -> p dc h", h=HQ)
            in_v = w_ap[:, ds(r * HQ, HQ)].rearrange("(dc p) h -> p dc h", p=P)
            if par is not None:
                out_v = wt[wn][r].rearrange(
                    "p (dc2 par h) -> p par dc2 h", par=2, h=HQ
                )[:, par]
                in_v = w_ap[:, ds(r * HQ, HQ)].rearrange(
                    "(dc2 par p) h -> p par dc2 h", par=2, p=P
                )[:, par]
            eng.dma_start(out=out_v, in_=in_v)

        w_issue("z", 0, nc.sync, par=0)
        w_issue("z", 0, nc.scalar, par=1)
        w_issue("h", 0, nc.sync, par=0)
        w_issue("h", 0, nc.scalar, par=1)
        for r in range(1, 4):
            w_issue("z", r, nc.sync)
            w_issue("h", r, nc.scalar)

        xts = {}

        def t_x(s):  # crossbar transpose, issued a full step ahead of use
            b, tci = step_bt(s)
            tiles = []
            for dc in range(DC):
                t_ = xt_p.tile([P, TC], BF16, tag=f"xt{dc}", name=f"xt_{s}_{dc}")
                nc.sync.dma_start_transpose(t_, x[b, ds(tci * TC, TC), ts(dc, P)])
                tiles.append(t_)
            xts[s] = tiles

        t_x(1)

        # Prologue PE work: HAM warmup junk, then step 0's x^T on the PE
        # (transpose to PSUM j-outer in two 4-dc waves so work starts as
        # each xn row-block lands; ACT/DVE alternate the PSUM->SBUF copies).
        xts[0] = []
        with tc.tile_pool(name="warm", bufs=1, space="PSUM") as warm_p, \
             tc.tile_pool(name="pxt", bufs=1, space="PSUM") as pxt_p, \
             tc.tile_pool(name="wdram", bufs=1, space="DRAM") as wdram_p:
            # HAM warmup. NOTE: keep this modest — a sustained ~9 us
            # full-width warmup was measured to push the whole run into a
            # lower power state (+50 us); absorbing the ~7-10 us HAM
            # ramp clamp pre-GEMM does not work.
            junk_ps = warm_p.tile([P, P], F32, name="junk_ps")
            NWARM = 30
            for i in range(NWARM):
                nc.tensor.matmul(
                    junk_ps, id_bf, id_bf, start=(i == 0), stop=(i == NWARM - 1)
                )
            junk_sb = consts.tile([P, P], F32, name="junk_sb")
            nc.vector.tensor_copy(junk_sb, junk_ps)
            junk_dr = wdram_p.tile([P, P], F32, name="junk_dr")
            nc.sync.dma_start(out=junk_dr, in_=junk_sb)

            for wave in range(2):
                pxts = [
                    pxt_p.tile([P, TC], BF16, tag=f"pxt{k}", name=f"pxt_{wave}_{k}")
                    for k in range(4)
                ]
                for j in range(TSUB):
                    for k in range(4):
                        dc = wave * 4 + k
                        nc.tensor.transpose(
                            pxts[k][:, ts(j, P)], xnt(j)[:, ts(dc, P)], id_bf
                        )
                for k in range(4):
                    dc = wave * 4 + k
                    xt_sb = xt_p.tile([P, TC], BF16, tag=f"xt{dc}", name=f"xt_0_{dc}")
                    if k % 2:
                        nc.scalar.copy(xt_sb, pxts[k])
                    else:
                        nc.vector.tensor_copy(xt_sb, pxts[k])
                    xts[0].append(xt_sb)

        # PSUM: 4 z + 4 h GEMM banks (prologue banks are re-used once the
        # ACT/DVE copies above have drained — before the first GEMM needs
        # them).
        pz_p = ctx.enter_context(tc.tile_pool(name="pz", bufs=4, space="PSUM"))
        ph_p = ctx.enter_context(tc.tile_pool(name="ph", bufs=4, space="PSUM"))

        scans = {}

        def gemm(s, hc, wn):
            pool = pz_p if wn == "z" else ph_p
            psum = pool.tile([P, TC], F32, tag="p" + wn, name=f"ps{wn}_{s}_{hc}")
            xt = xts[s]
            w_sb = wt[wn][hc // 2]
            for dc in range(DC):
                nc.tensor.matmul(
                    psum,
                    w_sb[:, ds(dc * HQ + (hc % 2) * P, P)],
                    xt[dc],
                    start=(dc == 0),
                    stop=(dc == DC - 1),
                )
            return psum

        def post(s, hc, psum_z, psum_h):
            b, tci = step_bt(s)
            # z before a: the DVE fuse (STT) only needs z, so emitting z
            # first lets the STT start one ACT-op (~0.7us) earlier; the
            # scan needs a only after the STT anyway.
            z_sb = azb_p.tile([P, TC], F32, tag="z", name=f"z_{s}_{hc}")
            nc.scalar.activation(
                z_sb, psum_z, AF.Sigmoid, bias=bz_sb[:, hc : hc + 1], scale=1.0
            )
            a_sb = azb_p.tile([P, TC], BF16, tag="a", name=f"a_{s}_{hc}")
            nc.scalar.activation(
                a_sb, psum_z, AF.Sigmoid, bias=nbz_sb[:, hc : hc + 1], scale=-1.0
            )
            bsc = azb_p.tile([P, TC], BF16, tag="b", name=f"b_{s}_{hc}")
            nc.vector.scalar_tensor_tensor(
                bsc, psum_h, bh_sb[:, hc : hc + 1], z_sb, op0=OP.add, op1=OP.mult
            )
            # bf16 scan output: the scan accumulator is fp32 in HW
            # regardless of out dtype, so only stored values round; bf16
            # halves the store bytes.
            if tci == 0:
                init = hp_sb[:, b * HC + hc : b * HC + hc + 1]
            else:
                init = scans[s - 1][hc][:, TC - 1 : TC]
            sc = scan_p.tile([P, TC], BF16, tag=f"sc{hc}", name=f"sc_{s}_{hc}")
            nc.vector.tensor_tensor_scan(sc, a_sb, bsc, init, op0=OP.mult, op1=OP.add)
            scans.setdefault(s, [None] * HC)[hc] = sc
            # Store the scan tile directly in [H, T] layout on the SP
            # HWDGE queue (host transposes). The store's scan-semaphore
            # wait must NOT sit on the ACT engine: it would serialize the
            # sigmoids (which drain the GEMM PSUM banks) behind the DVE
            # scan chain and stall the PE. The SP engine only issues the
            # next-next step's crossbar loads after these, which is safe
            # (exception: the terminal chain above, where ACT is done).
            nc.sync.dma_start(
                out=out[b, ds(hc * P, P), ds(tci * TC, TC)], in_=sc
            )

        def terminal(s, hc):
            # Last GEMM pair of the kernel: run it as two half-token
            # groups in separate PSUM tiles so the first half's whole
            # post chain (sigmoids -> STT -> scan) executes under the
            # second half's GEMM shadow; only a 256-token chain remains
            # after the final matmul (vs a full 512 one).
            b, tci = step_bt(s)
            HT = TC // 2
            w_z = wt["z"][hc // 2]
            w_h = wt["h"][hc // 2]
            prev_sc = None
            for half in range(2):
                ps = {}
                for wn, w_sb in (("z", w_z), ("h", w_h)):
                    pool = pz_p if wn == "z" else ph_p
                    psum = pool.tile(
                        [P, TC], F32, tag="p" + wn, name=f"t{wn}{half}"
                    )
                    for dc in range(DC):
                        nc.tensor.matmul(
                            psum[:, ds(0, HT)],
                            w_sb[:, ds(dc * HQ + (hc % 2) * P, P)],
                            xts[s][dc][:, ds(half * HT, HT)],
                            start=(dc == 0),
                            stop=(dc == DC - 1),
                        )
                    ps[wn] = psum
                z_sb = azb_p.tile([P, TC], F32, tag="z", name=f"tz_{half}")
                nc.scalar.activation(
                    z_sb[:, ds(0, HT)], ps["z"][:, ds(0, HT)], AF.Sigmoid,
                    bias=bz_sb[:, hc : hc + 1], scale=1.0,
                )
                a_sb = azb_p.tile([P, TC], BF16, tag="a", name=f"ta_{half}")
                nc.scalar.activation(
                    a_sb[:, ds(0, HT)], ps["z"][:, ds(0, HT)], AF.Sigmoid,
                    bias=nbz_sb[:, hc : hc + 1], scale=-1.0,
                )
                bsc = azb_p.tile([P, TC], BF16, tag="b", name=f"tb_{half}")
                nc.vector.scalar_tensor_tensor(
                    bsc[:, ds(0, HT)], ps["h"][:, ds(0, HT)],
                    bh_sb[:, hc : hc + 1], z_sb[:, ds(0, HT)],
                    op0=OP.add, op1=OP.mult,
                )
                sc_h = scan_p.tile([P, HT], BF16, tag=f"t7{half}", name=f"t7{half}")
                if half == 0:
                    init = scans[s - 1][hc][:, TC - 1 : TC]
                else:
                    init = prev_sc[:, HT - 1 : HT]
                nc.vector.tensor_tensor_scan(
                    sc_h, a_sb[:, ds(0, HT)], bsc[:, ds(0, HT)], init,
                    op0=OP.mult, op1=OP.add,
                )
                eng = nc.scalar if half == 0 else nc.sync
                eng.dma_start(
                    out=out[b, ds(hc * P, P), ds(tci * TC + half * HT, HT)],
                    in_=sc_h,
                )
                prev_sc = sc_h

        # --- steps -------------------------------------------------------
        for s in range(NSTEP):
            if s == 0:
                # GEMMs in z/h quarter-pairs so they track the W quarter
                # DMAs landing on the two queues.
                for r in range(4):
                    pzs = [gemm(0, 2 * r + i, "z") for i in range(2)]
                    phs = [gemm(0, 2 * r + i, "h") for i in range(2)]
                    for i in range(2):
                        post(0, 2 * r + i, pzs[i], phs[i])
            else:
                if s + 1 < NSTEP:
                    t_x(s + 1)
                for hc in range(HC):
                    if s == NSTEP - 1 and hc == HC - 1:
                        terminal(s, hc)
                        continue
                    psum_z = gemm(s, hc, "z")
                    psum_h = gemm(s, hc, "h")
                    post(s, hc, psum_z, psum_h)
                if s - 2 in scans:
                    del scans[s - 2]


def build():
    if "nc" in _CACHE:
        return _CACHE["nc"]
    nc = bacc.Bacc(
        "TRN2", target_bir_lowering=False, debug=False, num_devices=NCORES
    )
    x = nc.dram_tensor("x", [B_LOC, T, D], BF16, kind="ExternalInput").ap()
    h0 = nc.dram_tensor("h0", [B_LOC, H], F32, kind="ExternalInput").ap()
    wzT = nc.dram_tensor("wzT", [D, H], BF16, kind="ExternalInput").ap()
    bz = nc.dram_tensor("bz", [H], F32, kind="ExternalInput").ap()
    whT = nc.dram_tensor("whT", [D, H], BF16, kind="ExternalInput").ap()
    bh = nc.dram_tensor("bh", [H], F32, kind="ExternalInput").ap()
    out = nc.dram_tensor("out", [B_LOC, H, T], BF16, kind="ExternalOutput").ap()
    with tile.TileContext(nc) as tctx:
        _mingru_tile(tctx, out, x, h0, wzT, bz, whT, bh)
    nc.compile()
    _CACHE["nc"] = nc
    return nc


def make_in_maps(x, h_prev, W_z, b_z, W_h, b_h):
    x = np.asarray(x, dtype=np.float32).astype(ml_dtypes.bfloat16)
    h_prev = np.ascontiguousarray(np.asarray(h_prev, dtype=np.float32))
    wzT = np.asarray(W_z, dtype=np.float32).T.astype(ml_dtypes.bfloat16)
    whT = np.asarray(W_h, dtype=np.float32).T.astype(ml_dtypes.bfloat16)
    b_z = np.ascontiguousarray(np.asarray(b_z, dtype=np.float32))
    b_h = np.ascontiguousarray(np.asarray(b_h, dtype=np.float32))
    in_maps = []
    for c in range(NCORES):
        sl = slice(c * B_LOC, (c + 1) * B_LOC)
        in_maps.append(
            {
                "x": np.ascontiguousarray(x[sl]),
                "h0": h_prev[sl],
                "wzT": wzT,
                "bz": b_z,
                "whT": whT,
                "bh": b_h,
            }
        )
    return in_maps


def kernel(x, h_prev, W_z, b_z, W_h, b_h, trace=False):
    nc = build()
    in_maps = make_in_maps(x, h_prev, W_z, b_z, W_h, b_h)
    res = run_bass_kernel_spmd(
        nc, in_maps, core_ids=list(range(NCORES)), trace=trace
    )
    # Device output is [B_loc, H, T] bf16; transpose to (B, T, H) f32 here.
    out = np.concatenate(
        [
            np.asarray(r["out"]).astype(np.float32).transpose(0, 2, 1)
            for r in res.results
        ],
        axis=0,
    )
    if trace:
        _CACHE["last_results"] = res
    return out
